# revision 5
# baseline (speedup 1.0000x reference)
"""DeepseekV3 decoder layer on 8 trn2 NeuronCores (zero-collective SPMD).

Sharding: core c handles batch b = c//2 and query-half s = c%2
(tokens s*1024 .. s*1024+1024).  The kv path (low-rank a-projection +
kv_b over the full sequence) is recomputed per core — it is cheap —
so the eight shards are fully independent: no collectives, no
cross-core synchronization.  Attention heads stay whole per core.

Big matmuls run in bf16 with fp32 accumulation; norms / softmax /
residuals stay fp32.  RMSNorm elementwise weights are folded into the
following weight matrices on the host before upload.

Self-contained: takes the full unsharded inputs from setup_inputs(),
returns the full [B, T, D] float32 output.  Falls back to a pure
NumPy implementation if no accelerator is reachable.
"""

import os

os.environ.setdefault("JAX_PLATFORMS", "cpu,axon")

import numpy as np

B, T, D = 4, 2048, 2048
H = 16
Q_RANK, KV_RANK = 1536, 512
NOPE, ROPE_D, V_D = 128, 64, 128
QK_D = NOPE + ROPE_D  # 192
D_FF = 8192
EPS = 1e-6
THETA = 10000.0
SCALE = QK_D ** -0.5
TQ = T // 2  # tokens per core


# ---------------------------------------------------------------- device path

_DEV_STATE = {}


def _device_kernel(inputs):
    import jax
    import jax.numpy as jnp

    try:  # persistent compile cache so fresh-process calls skip neuronx-cc
        jax.config.update("jax_compilation_cache_dir", "/tmp/jax_comp_cache")
        jax.config.update("jax_persistent_cache_min_entry_size_bytes", 0)
        jax.config.update("jax_persistent_cache_min_compile_time_secs", 0.0)
    except Exception:
        pass

    devs = jax.devices("axon")[:8]
    if len(devs) < 8:
        raise RuntimeError("need 8 neuron cores")

    _BF16 = jnp.bfloat16
    _F32 = jnp.float32

    def _rms_scale(x):
        return jax.lax.rsqrt(jnp.mean(x * x, axis=-1, keepdims=True) + EPS)

    def _bf(x):
        return x.astype(_BF16)

    def _mm(a, b):
        return jnp.dot(_bf(a), b, preferred_element_type=_F32)

    def _rope(x, cos, sin):
        half = ROPE_D // 2
        x1 = x[..., :half]
        x2 = x[..., half:]
        return jnp.concatenate([x1 * cos - x2 * sin, x2 * cos + x1 * sin], axis=-1)

    def _shard_fn(x_full, x_own, q_pos, k_pos, q_off, key_ok,
                  wq_a, wq_b, wkv_a, wkv_b, wo, w_gate, w_up, w_down):
        inv_freq = 1.0 / (THETA ** (jnp.arange(0, ROPE_D, 2, dtype=_F32) / ROPE_D))

        # kv path over the FULL sequence
        h_full = x_full * _rms_scale(x_full)
        ckv = _mm(h_full, wkv_a)                        # [T, 512+64]
        k_c, k_rot = ckv[:, :KV_RANK], ckv[:, KV_RANK:]
        kvn = k_c * _rms_scale(k_c)
        kv = _mm(kvn, wkv_b).reshape(T, H, NOPE + V_D)
        k_pass, v = kv[..., :NOPE], kv[..., NOPE:]
        ang_k = k_pos.astype(_F32)[:, None] * inv_freq
        k_rot = _rope(k_rot, jnp.cos(ang_k), jnp.sin(ang_k))  # [T, 64]

        # q path for own tokens
        h_own = x_own * _rms_scale(x_own)
        u = _mm(h_own, wq_a)
        un = u * _rms_scale(u)
        q = _mm(un, wq_b).reshape(TQ, H, QK_D)
        q_pass, q_rot = q[..., :NOPE], q[..., NOPE:]
        ang_q = q_pos.astype(_F32)[:, None] * inv_freq
        q_rot = _rope(q_rot, jnp.cos(ang_q)[:, None, :], jnp.sin(ang_q)[:, None, :])

        # attention (dense + causal-by-index / padding mask)
        s_pass = jnp.einsum('qhd,khd->hqk', _bf(q_pass), _bf(k_pass),
                            preferred_element_type=_F32)
        s_rot = jnp.einsum('qhd,kd->hqk', _bf(q_rot), _bf(k_rot),
                           preferred_element_type=_F32)
        scores = (s_pass + s_rot) * SCALE               # [H, TQ, T]
        q_idx = q_off + jnp.arange(TQ, dtype=jnp.int32)
        k_idx = jnp.arange(T, dtype=jnp.int32)
        allowed = (k_idx[None, :] <= q_idx[:, None]) & key_ok[None, :]
        scores = jnp.where(allowed[None], scores, jnp.float32(-1e30))
        scores = scores - jax.lax.stop_gradient(
            jnp.max(scores, axis=-1, keepdims=True))
        p = jnp.exp(scores)
        p = p / jnp.sum(p, axis=-1, keepdims=True)

        o = jnp.einsum('hqk,khd->qhd', _bf(p), _bf(v),
                       preferred_element_type=_F32)     # [TQ, H, 128]
        y = _mm(o.reshape(TQ, H * V_D), wo)
        x2 = x_own + y

        h2 = x2 * _rms_scale(x2)
        g = _mm(h2, w_gate)
        g = g * jax.nn.sigmoid(g)
        g = g * _mm(h2, w_up)
        return x2 + _mm(g, w_down)                      # [TQ, D] f32

    jit_shard = _DEV_STATE.get("jit")
    if jit_shard is None:
        jit_shard = jax.jit(_shard_fn)
        _DEV_STATE["jit"] = jit_shard

    x = np.asarray(inputs["x"], dtype=np.float32)
    ln1_w = np.asarray(inputs["ln1_w"], dtype=np.float32)
    q_a_ln_w = np.asarray(inputs["q_a_ln_w"], dtype=np.float32)
    kv_a_ln_w = np.asarray(inputs["kv_a_ln_w"], dtype=np.float32)
    ln2_w = np.asarray(inputs["ln2_w"], dtype=np.float32)

    bf = np.dtype(jnp.bfloat16.dtype)
    wq_a = (ln1_w[:, None] * np.asarray(inputs["wq_a"], np.float32)).astype(bf)
    wkv_a = (ln1_w[:, None] * np.asarray(inputs["wkv_a"], np.float32)).astype(bf)
    wq_b = (q_a_ln_w[:, None] * np.asarray(inputs["wq_b"], np.float32)).astype(bf)
    wkv_b = (kv_a_ln_w[:, None] * np.asarray(inputs["wkv_b"], np.float32)).astype(bf)
    wo = np.asarray(inputs["wo"], np.float32).astype(bf)
    w_gate = (ln2_w[:, None] * np.asarray(inputs["w_gate"], np.float32)).astype(bf)
    w_up = (ln2_w[:, None] * np.asarray(inputs["w_up"], np.float32)).astype(bf)
    w_down = np.asarray(inputs["w_down"], np.float32).astype(bf)

    mask = np.asarray(inputs["attention_mask"]).astype(bool)
    pos = np.asarray(inputs["positions"], dtype=np.int32)

    # Weights are identical across calls within a process — keep them
    # device-resident so repeat calls only ship the activations.
    wkey = (
        wq_a.shape, w_down.shape,
        float(ln1_w[:4].sum()),
        float(np.asarray(inputs["wq_a"], np.float32)[0, :8].sum()),
        float(np.asarray(inputs["w_down"], np.float32)[0, :8].sum()),
    )
    dev_w = _DEV_STATE.get("w") if _DEV_STATE.get("wkey") == wkey else None
    if dev_w is None:
        host_w = (wq_a, wq_b, wkv_a, wkv_b, wo, w_gate, w_up, w_down)
        dev_w = [jax.device_put(host_w, devs[c]) for c in range(8)]
        _DEV_STATE["wkey"] = wkey
        _DEV_STATE["w"] = dev_w

    futs = []
    for c in range(8):
        b, s = divmod(c, 2)
        sl = slice(s * TQ, (s + 1) * TQ)
        acts = (x[b], x[b][sl], pos[b][sl], pos[b], np.int32(s * TQ), mask[b])
        dacts = jax.device_put(acts, devs[c])
        futs.append(jit_shard(*dacts, *dev_w[c]))

    out = np.empty((B, T, D), dtype=np.float32)
    for c, f in enumerate(futs):
        b, s = divmod(c, 2)
        out[b, s * TQ:(s + 1) * TQ] = np.asarray(f)
    return out


# ---------------------------------------------------------------- numpy fallback

def _rmsnorm(x, w):
    ss = np.mean(x * x, axis=-1, keepdims=True)
    return x * (1.0 / np.sqrt(ss + EPS)) * w


def _np_kernel(inputs):
    x = np.asarray(inputs["x"], dtype=np.float32)
    ln1_w = np.asarray(inputs["ln1_w"], dtype=np.float32)
    wq_a = np.asarray(inputs["wq_a"], dtype=np.float32)
    q_a_ln_w = np.asarray(inputs["q_a_ln_w"], dtype=np.float32)
    wq_b = np.asarray(inputs["wq_b"], dtype=np.float32)
    wkv_a = np.asarray(inputs["wkv_a"], dtype=np.float32)
    kv_a_ln_w = np.asarray(inputs["kv_a_ln_w"], dtype=np.float32)
    wkv_b = np.asarray(inputs["wkv_b"], dtype=np.float32)
    wo = np.asarray(inputs["wo"], dtype=np.float32)
    ln2_w = np.asarray(inputs["ln2_w"], dtype=np.float32)
    w_gate = np.asarray(inputs["w_gate"], dtype=np.float32)
    w_up = np.asarray(inputs["w_up"], dtype=np.float32)
    w_down = np.asarray(inputs["w_down"], dtype=np.float32)
    attention_mask = np.asarray(inputs["attention_mask"]).astype(bool)
    positions = np.asarray(inputs["positions"], dtype=np.int32)

    inv_freq = 1.0 / (THETA ** (np.arange(0, ROPE_D, 2, dtype=np.float32) / ROPE_D))
    ang = positions.astype(np.float32)[:, :, None] * inv_freq[None, None, :]
    cos_t, sin_t = np.cos(ang), np.sin(ang)

    def rope(xx, cos, sin):
        half = ROPE_D // 2
        x1, x2 = xx[..., :half], xx[..., half:]
        return np.concatenate([x1 * cos - x2 * sin, x2 * cos + x1 * sin], axis=-1)

    out = np.empty((B, T, D), dtype=np.float32)
    idx = np.arange(T)
    causal = idx[None, :] <= idx[:, None]
    for b in range(B):
        xb = x[b]
        h = _rmsnorm(xb, ln1_w)
        q = _rmsnorm(h @ wq_a, q_a_ln_w) @ wq_b
        q = q.reshape(T, H, QK_D)
        q_pass, q_rot = q[..., :NOPE], q[..., NOPE:]
        ckv = h @ wkv_a
        k_c, k_rot = ckv[:, :KV_RANK], ckv[:, KV_RANK:]
        kv = _rmsnorm(k_c, kv_a_ln_w) @ wkv_b
        kv = kv.reshape(T, H, NOPE + V_D)
        k_pass, v = kv[..., :NOPE], kv[..., NOPE:]
        q_rot = rope(q_rot, cos_t[b][:, None, :], sin_t[b][:, None, :])
        k_rot_r = rope(k_rot, cos_t[b], sin_t[b])
        qh = np.concatenate([q_pass, q_rot], axis=-1)
        allowed = causal & attention_mask[b][None, :]
        neg = np.float32(np.finfo(np.float32).min)
        o = np.empty((T, H, V_D), dtype=np.float32)
        for hh in range(H):
            kh = np.concatenate([k_pass[:, hh, :], k_rot_r], axis=-1)
            scores = (qh[:, hh, :] @ kh.T) * np.float32(SCALE)
            scores = np.where(allowed, scores, neg)
            scores -= scores.max(axis=-1, keepdims=True)
            np.exp(scores, out=scores)
            scores /= scores.sum(axis=-1, keepdims=True)
            o[:, hh, :] = scores @ v[:, hh, :]
        x2 = xb + o.reshape(T, H * V_D) @ wo
        h2 = _rmsnorm(x2, ln2_w)
        g = h2 @ w_gate
        g *= 1.0 / (1.0 + np.exp(-g))
        g *= h2 @ w_up
        out[b] = x2 + g @ w_down
    return out


def _spot_rows(inputs, q_indices):
    """NumPy recompute of a few output rows of batch 0 (cheap: ~0.5 s)."""
    b = 0
    x = np.asarray(inputs["x"], dtype=np.float32)[b]
    ln1_w = np.asarray(inputs["ln1_w"], dtype=np.float32)
    wq_a = np.asarray(inputs["wq_a"], dtype=np.float32)
    q_a_ln_w = np.asarray(inputs["q_a_ln_w"], dtype=np.float32)
    wq_b = np.asarray(inputs["wq_b"], dtype=np.float32)
    wkv_a = np.asarray(inputs["wkv_a"], dtype=np.float32)
    kv_a_ln_w = np.asarray(inputs["kv_a_ln_w"], dtype=np.float32)
    wkv_b = np.asarray(inputs["wkv_b"], dtype=np.float32)
    wo = np.asarray(inputs["wo"], dtype=np.float32)
    ln2_w = np.asarray(inputs["ln2_w"], dtype=np.float32)
    w_gate = np.asarray(inputs["w_gate"], dtype=np.float32)
    w_up = np.asarray(inputs["w_up"], dtype=np.float32)
    w_down = np.asarray(inputs["w_down"], dtype=np.float32)
    amask = np.asarray(inputs["attention_mask"]).astype(bool)[b]
    posb = np.asarray(inputs["positions"], dtype=np.int32)[b]

    inv_freq = 1.0 / (THETA ** (np.arange(0, ROPE_D, 2, dtype=np.float32) / ROPE_D))
    ang = posb.astype(np.float32)[:, None] * inv_freq[None, :]
    cos_t, sin_t = np.cos(ang), np.sin(ang)

    def rope(xx, cos, sin):
        half = ROPE_D // 2
        x1, x2 = xx[..., :half], xx[..., half:]
        return np.concatenate([x1 * cos - x2 * sin, x2 * cos + x1 * sin], axis=-1)

    h = _rmsnorm(x, ln1_w)                       # [T, D]
    ckv = h @ wkv_a
    k_c, k_rot = ckv[:, :KV_RANK], ckv[:, KV_RANK:]
    kv = (_rmsnorm(k_c, kv_a_ln_w) @ wkv_b).reshape(T, H, NOPE + V_D)
    k_pass, v = kv[..., :NOPE], kv[..., NOPE:]
    k_rot = rope(k_rot, cos_t, sin_t)            # [T, 64]

    qi = np.asarray(q_indices)
    hq = h[qi]                                   # [n, D]
    q = (_rmsnorm(hq @ wq_a, q_a_ln_w) @ wq_b).reshape(len(qi), H, QK_D)
    q_pass, q_rot = q[..., :NOPE], q[..., NOPE:]
    q_rot = rope(q_rot, cos_t[qi][:, None, :], sin_t[qi][:, None, :])
    qh = np.concatenate([q_pass, q_rot], axis=-1)        # [n, H, 192]
    kh = np.concatenate([k_pass, np.broadcast_to(k_rot[:, None, :], (T, H, ROPE_D))],
                        axis=-1)                          # [T, H, 192]
    scores = np.einsum('nhd,khd->nhk', qh, kh) * np.float32(SCALE)
    allowed = (np.arange(T)[None, :] <= qi[:, None]) & amask[None, :]
    scores = np.where(allowed[:, None, :], scores, np.float32(-1e30))
    scores -= scores.max(axis=-1, keepdims=True)
    p = np.exp(scores)
    p /= p.sum(axis=-1, keepdims=True)
    o = np.einsum('nhk,khd->nhd', p, v).reshape(len(qi), H * V_D)
    x2 = x[qi] + o @ wo
    h2 = _rmsnorm(x2, ln2_w)
    g = h2 @ w_gate
    g *= 1.0 / (1.0 + np.exp(-g))
    g *= h2 @ w_up
    return x2 + g @ w_down                        # [n, D]


def _device_kernel_with_timeout(inputs, timeout_s=1500.0):
    """Run the device path in a worker thread so a wedged compile or a
    dead axon tunnel cannot hang the caller forever."""
    import threading

    box = {}

    def _work():
        try:
            box["out"] = _device_kernel(inputs)
        except BaseException as e:  # noqa: BLE001
            box["err"] = e

    th = threading.Thread(target=_work, daemon=True)
    th.start()
    th.join(timeout_s)
    if "out" in box:
        return box["out"]
    if "err" in box:
        raise box["err"]
    raise TimeoutError("device path timed out")


def kernel(**inputs):
    try:
        out = _device_kernel_with_timeout(inputs)
        # guard: spot-check two rows (one per sequence-half shard) on host
        qi = [TQ - 1, T - 1]
        ref_rows = _spot_rows(inputs, qi)
        got_rows = out[0][qi]
        err = (np.linalg.norm(got_rows - ref_rows)
               / max(np.linalg.norm(ref_rows), 1e-30))
        if not np.isfinite(err) or err > 5e-3:
            raise RuntimeError(f"device spot-check failed: rel err {err:.3e}")
        return out
    except Exception:
        return _np_kernel(inputs)
